# revision 12
# baseline (speedup 1.0000x reference)
"""Channel-attention (CAM) kernel for Trainium2, 8 NeuronCores.

Reference computation (per batch b):
    A   = x[b].reshape(L, C)            # L = 48^3 = 110592, C = 256
    G   = A^T A                          # [C, C] Gram matrix
    S   = softmax(G, axis=-1)
    out = gamma * (A @ S) + x[b]

Algebraic fold: out = A @ (gamma*S + I) since A @ I == x.  This removes
the residual add AND the second read of x: HBM traffic is the floor
(read 28.3 MB + write 28.3 MB per core).  A^T (bf16) stays resident in
SBUF between the phases.

Collective-latency engineering (the dominant cost): an AllReduce here
completes ~16-20 us after the LAST rank triggers it, and the runtime
starts the 8 cores with a 20-30 us skew, so the first collective also
absorbs that skew in its entry barrier.  Batch-0's Gram is therefore
all-reduced in TWO pieces: AR0a covers supertiles 0-4 and triggers at
~35 us (soaking the start skew while the read stream continues), AR0b
covers 5-8 and triggers as soon as AR0a completes (ranks now aligned,
so it costs only the ~16-20 us floor); G_0 = out_a + out_b summed
locally inside the softmax.  AR1 (batch 1, one piece) then rides the
aligned ranks too and overlaps batch-0's Y matmuls + stores.

The gpsimd queue carries ONLY Gram staging + the collectives (a
collective blocks its queue until completion, so nothing else may sit
behind one).  All x loads are plain fp32 halves on the sync queue,
converted to bf16 on the vector engine.  Transposes for 3 supertiles
are deferred into the AR wait gaps so phase-1 PE work stays under the
DMA read stream.

Engine queues (FIFO, emission order = issue order):
  sync    = gamma, all x loads, output stores
  gpsimd  = Gram staging copies + staging DMAs + AR0a/AR0b/AR1
  scalar  = half the A^T/Y drains, softmax activations, cc_out loads
  vector  = bf16 converts, other drains, softmax vector ops
"""

import numpy as np
from contextlib import ExitStack

import concourse.bass as bass
import concourse.tile as tile
from concourse import bacc, mybir
from concourse.bass import ts
from concourse.bass_utils import run_bass_kernel_spmd
from concourse.masks import make_identity

F32 = mybir.dt.float32
BF16 = mybir.dt.bfloat16
AF = mybir.ActivationFunctionType

N_CORES = 8
B = 2
L = 48 * 48 * 48          # 110592
C = 256
L_SH = L // N_CORES       # 13824 rows per core per batch
ROWS = B * L_SH           # 27648 rows per core
P = 128
RPP = 12                  # rows per partition per supertile
HPP = RPP // 2            # rows per partition per half-supertile
SROWS = P * RPP           # 1536 rows per supertile
HROWS = P * HPP           # 768 rows per half-supertile
SPB = L_SH // SROWS       # 9 supertiles per batch
S_TOT = B * SPB           # 18 supertiles per core
SPLIT0 = 3                # supertiles 0..2 -> AR0a, 3..8 -> AR0b

DEFERRED = {7, 8, 16, 17}  # transposes run in the AR wait gaps

_CACHE: dict = {}


def _build():
    nc = bacc.Bacc(
        "TRN2", target_bir_lowering=False, debug=False, num_devices=N_CORES
    )
    x_dram = nc.dram_tensor("x", [ROWS, C], F32, kind="ExternalInput")
    g_dram = nc.dram_tensor("gamma", [1, 1], F32, kind="ExternalInput")
    o_dram = nc.dram_tensor("out", [ROWS, C], F32, kind="ExternalOutput")
    # three collective groups: 0 = b0 supertiles 0..4, 1 = b0 5..8, 2 = b1
    cc_in = [
        nc.dram_tensor(f"cc_in{g}", [2 * P, C], BF16, kind="Internal")
        for g in range(3)
    ]
    cc_out = [
        nc.dram_tensor(
            f"cc_out{g}", [2 * P, C], BF16, kind="Internal",
            addr_space="Shared",
        )
        for g in range(3)
    ]
    X, GAM, OUT = x_dram.ap(), g_dram.ap(), o_dram.ap()

    def x_half(s, h):
        # partition p holds rows s*SROWS + h*HROWS + p*HPP + (0..HPP-1)
        r0 = s * SROWS + h * HROWS
        return X[r0 : r0 + HROWS, :].rearrange("(p j) c -> p j c", j=HPP)

    def x_super(s):
        # same per-half row mapping, both halves in one DMA
        return X[ts(s, SROWS), :].rearrange(
            "(h p j) c -> p h j c", h=2, j=HPP
        )

    def o_half(s, h):
        r0 = s * SROWS + h * HROWS
        return OUT[r0 : r0 + HROWS, :].rearrange("(p j) c -> p j c", j=HPP)

    def grp(s):
        return 0 if s < SPLIT0 else (1 if s < SPB else 2)

    GRP_BOUNDS = {0: (0, SPLIT0 - 1), 1: (SPLIT0, SPB - 1), 2: (SPB, S_TOT - 1)}

    with tile.TileContext(nc) as tc, ExitStack() as octx:
        constp = octx.enter_context(tc.tile_pool(name="const", bufs=1))
        ident = constp.tile([P, P], BF16, name="ident", tag="ident")
        make_identity(nc, ident[:])
        gam_sb = constp.tile([1, 1], F32, name="gam_sb", tag="gam_sb")
        nc.sync.dma_start(gam_sb[:], GAM[:, :])
        gam_bc = constp.tile([P, 1], F32, name="gam_bc", tag="gam_bc")
        nc.gpsimd.partition_broadcast(gam_bc[:], gam_sb[:])
        # m_bf[2b+q] = gamma * softmax(G_b)[qP:(q+1)P, :] + I-block
        m_bf = [
            constp.tile([P, C], BF16, name=f"mbf{i}", tag=f"mbf{i}")
            for i in range(4)
        ]

        attp = octx.enter_context(tc.tile_pool(name="att", bufs=S_TOT))
        xbsp = octx.enter_context(tc.tile_pool(name="xbs", bufs=3))   # stream
        xbhp = octx.enter_context(tc.tile_pool(name="xbh", bufs=4))   # held
        xfp = octx.enter_context(tc.tile_pool(name="xf", bufs=4))     # fp32
        otp = octx.enter_context(tc.tile_pool(name="ot", bufs=3))
        gsp = octx.enter_context(tc.tile_pool(name="gs", bufs=6))
        smp = octx.enter_context(tc.tile_pool(name="smx", bufs=2))
        psg = octx.enter_context(tc.tile_pool(name="psg", bufs=1, space="PSUM"))
        pst = octx.enter_context(tc.tile_pool(name="pst", bufs=2, space="PSUM"))
        psy = octx.enter_context(tc.tile_pool(name="psy", bufs=3, space="PSUM"))

        # one [P, 2, C] accumulator pair per collective group
        g_pair = [
            psg.tile([P, 2, C], F32, name=f"gpr{g}", tag=f"gpr{g}")
            for g in range(3)
        ]
        att: dict = {}
        xbs: dict = {}

        def xb_tile(s):
            pool = xbhp if s in DEFERRED else xbsp
            xb = pool.tile([P, RPP, C], BF16, name="xb", tag="xb")
            xbs[s] = xb
            return xb

        def gram(s, h):
            g, xb = grp(s), xbs[s]
            s_first, s_last = GRP_BOUNDS[g]
            for jj in range(HPP):
                j = h * HPP + jj
                first = s == s_first and j == 0
                last = s == s_last and j == RPP - 1
                nc.tensor.matmul(
                    g_pair[g][:, 0, :], xb[:, j, 0:P], xb[:, j, :],
                    start=first, stop=last,
                )
                nc.tensor.matmul(
                    g_pair[g][:, 1, :], xb[:, j, P:C], xb[:, j, :],
                    start=first, stop=last,
                )

        def tp_work(s, jplo, jphi):
            xb = xbs[s]
            if s in att:
                at = att[s]
            else:
                at = attp.tile([P, RPP, 2, P], BF16, name="at", tag="at")
                att[s] = at
            for jp in range(jplo, jphi):
                tpt = pst.tile([P, 2, 2, P], F32, name="tp", tag="tp")
                for dj in range(2):
                    j = 2 * jp + dj
                    for q in range(2):
                        nc.tensor.matmul(
                            tpt[:, dj, q, :], xb[:, j, ts(q, P)], ident[:],
                            start=True, stop=True,
                        )
                dst = at[:, ts(jp, 2), :, :]
                if jp == 3:
                    nc.vector.tensor_copy(dst, tpt[:])
                else:
                    nc.scalar.activation(dst, tpt[:], AF.Copy)

        def stage_and_ar(g):
            # staging copies + DMAs + collective all on the gpsimd queue:
            # nothing else lives there, so triggers are never queue-delayed
            for m in range(2):
                gsb = gsp.tile([P, C], BF16, name="gsb", tag="gsb")
                nc.vector.tensor_copy(gsb[:], g_pair[g][:, m, :])
                nc.gpsimd.dma_start(cc_in[g].ap()[ts(m, P), :], gsb[:])
            nc.gpsimd.collective_compute(
                "AllReduce",
                mybir.AluOpType.add,
                replica_groups=[list(range(N_CORES))],
                ins=[cc_in[g].ap()[:, :]],
                outs=[cc_out[g].ap()[:, :]],
            )

        def softmax(b):
            # b0: G = cc_out0 + cc_out1; b1: G = cc_out2
            groups = [0, 1] if b == 0 else [2]
            for m in range(2):
                i = 2 * b + m
                gfs = []
                for g in groups:
                    gf = smp.tile([P, C], BF16, name=f"gf{g}", tag=f"gf{g}")
                    nc.scalar.dma_start(gf[:], cc_out[g].ap()[ts(m, P), :])
                    gfs.append(gf)
                if len(gfs) == 2:
                    gt = smp.tile([P, C], F32, name="gt", tag="gt")
                    nc.vector.tensor_add(gt[:], gfs[0][:], gfs[1][:])
                else:
                    gt = gfs[0]
                nmx = smp.tile([P, 1], F32, name="nmx", tag="nmx")
                nc.vector.tensor_reduce(
                    nmx[:], gt[:],
                    axis=mybir.AxisListType.X,
                    op=mybir.AluOpType.max,
                    negate=True,
                )
                ex = smp.tile([P, C], F32, name="ex", tag="ex")
                ssum = smp.tile([P, 1], F32, name="ssum", tag="ssum")
                nc.scalar.activation(
                    ex[:], gt[:], AF.Exp, bias=nmx[:], scale=1.0,
                    accum_out=ssum[:],
                )
                inv = smp.tile([P, 1], F32, name="inv", tag="inv")
                nc.vector.reciprocal(inv[:], ssum[:])
                sc = smp.tile([P, 1], F32, name="sc", tag="sc")
                nc.vector.tensor_mul(sc[:], inv[:], gam_bc[:])
                nc.scalar.activation(m_bf[i][:], ex[:], AF.Copy, scale=sc[:])
                # fold the residual: M = gamma*S + I (diagonal block m)
                nc.vector.tensor_add(
                    m_bf[i][:, ts(m, P)], m_bf[i][:, ts(m, P)], ident[:]
                )

        def ywork(s):
            b, at = s // SPB, att[s]
            for h in range(2):
                ot = otp.tile([P, HPP, C], F32, name="ot", tag="ot")
                for jj in range(HPP // 2):
                    jp = h * (HPP // 2) + jj
                    y = psy.tile([P, 2, C], F32, name="y", tag="y")
                    for q in range(2):
                        j = 2 * jp + q
                        nc.tensor.matmul(
                            y[:, q, :], at[:, j, 0, :], m_bf[2 * b][:],
                            start=True, stop=False,
                        )
                        nc.tensor.matmul(
                            y[:, q, :], at[:, j, 1, :], m_bf[2 * b + 1][:],
                            start=False, stop=True,
                        )
                    dst = ot[:, ts(jj, 2), :]
                    if jj == 1:
                        nc.scalar.activation(dst, y[:], AF.Copy)
                    else:
                        nc.vector.tensor_copy(dst, y[:])
                nc.sync.dma_start(o_half(s, h), ot[:])

        # ---------------- phase 1: batch 0 (sync fp32 + converts) -------
        for s in range(SPB):
            xb = xb_tile(s)
            for h in range(2):
                xf = xfp.tile([P, HPP, C], F32, name="xf", tag="xf")
                nc.sync.dma_start(xf[:], x_half(s, h))
                nc.vector.tensor_copy(xb[:, ts(h, HPP), :], xf[:])
                gram(s, h)
                if s not in DEFERRED:
                    tp_work(s, h * (HPP // 2), (h + 1) * (HPP // 2))
            if s == SPLIT0 - 1:
                stage_and_ar(0)     # AR0a: triggers ~25us, soaks start skew

        # ---------------- phase 1: batch 1 (gpsimd cast loads) ----------
        # emitted AFTER AR0a on the gpsimd queue: the queue unblocks at
        # AR0a's ENTRY (a globally synced event), so every core's batch-1
        # read stream starts in lockstep and AR1's staging - and therefore
        # its entry barrier - no longer re-pays the start skew.
        for s in range(SPB, S_TOT):
            xb = xb_tile(s)
            nc.gpsimd.dma_start(                    # SWDGE cast f32->bf16
                xb[:].rearrange("p (h j) c -> p h j c", h=2), x_super(s)
            )
            for h in range(2):
                gram(s, h)
                if s not in DEFERRED:
                    tp_work(s, h * (HPP // 2), (h + 1) * (HPP // 2))
            if s == SPB:
                stage_and_ar(1)     # AR0b: rides AR0a's synced entry
            if s == S_TOT - 1:
                stage_and_ar(2)     # AR1: staging now skew-free

        # ---------------- phase 2 ----------------
        tp_work(7, 0, RPP // 2)     # fills the AR0b wait gap
        tp_work(8, 0, RPP // 2)
        softmax(0)
        for s in range(SPB):
            ywork(s)
        tp_work(16, 0, RPP // 2)    # fills the AR1 wait gap
        tp_work(17, 0, RPP // 2)
        softmax(1)
        for s in range(SPB, S_TOT):
            ywork(s)

    nc.compile()
    return nc


def _get_nc():
    if "nc" not in _CACHE:
        _CACHE["nc"] = _build()
    return _CACHE["nc"]


def kernel(x: np.ndarray, gamma: np.ndarray, **_kw) -> np.ndarray:
    nc = _get_nc()
    x = np.asarray(x, dtype=np.float32)
    orig_shape = x.shape
    x3 = x.reshape(B, L, C)
    gam = np.asarray(gamma, dtype=np.float32).reshape(1, 1)
    in_maps = []
    for k in range(N_CORES):
        shard = np.ascontiguousarray(
            x3[:, k * L_SH : (k + 1) * L_SH, :]
        ).reshape(ROWS, C)
        in_maps.append({"x": shard, "gamma": gam})
    res = run_bass_kernel_spmd(nc, in_maps, core_ids=list(range(N_CORES)))
    out = np.empty((B, L, C), dtype=np.float32)
    for k in range(N_CORES):
        out[:, k * L_SH : (k + 1) * L_SH, :] = res.results[k]["out"].reshape(
            B, L_SH, C
        )
    return out.reshape(orig_shape)


# revision 13
# speedup vs baseline: 1.0339x; 1.0339x over previous
"""Channel-attention (CAM) kernel for Trainium2, 8 NeuronCores.

Reference computation (per batch b):
    A   = x[b].reshape(L, C)            # L = 48^3 = 110592, C = 256
    G   = A^T A                          # [C, C] Gram matrix
    S   = softmax(G, axis=-1)
    out = gamma * (A @ S) + x[b]

Algebraic fold: out = A @ (gamma*S + I) since A @ I == x.  This removes
the residual add AND the second read of x: HBM traffic is the floor
(read 28.3 MB + write 28.3 MB per core).  A^T (bf16) stays resident in
SBUF between the phases.

Collective-latency engineering (the dominant cost): an AllReduce here
completes ~16-20 us after the LAST rank triggers it, and the runtime
starts the 8 cores with a 20-30 us skew, so the first collective also
absorbs that skew in its entry barrier.  Batch-0's Gram is therefore
all-reduced in TWO pieces: AR0a covers supertiles 0-4 and triggers at
~35 us (soaking the start skew while the read stream continues), AR0b
covers 5-8 and triggers as soon as AR0a completes (ranks now aligned,
so it costs only the ~16-20 us floor); G_0 = out_a + out_b summed
locally inside the softmax.  AR1 (batch 1, one piece) then rides the
aligned ranks too and overlaps batch-0's Y matmuls + stores.

The gpsimd queue carries ONLY Gram staging + the collectives (a
collective blocks its queue until completion, so nothing else may sit
behind one).  All x loads are plain fp32 halves on the sync queue,
converted to bf16 on the vector engine.  Transposes for 3 supertiles
are deferred into the AR wait gaps so phase-1 PE work stays under the
DMA read stream.

Engine queues (FIFO, emission order = issue order):
  sync    = gamma, all x loads, output stores
  gpsimd  = Gram staging copies + staging DMAs + AR0a/AR0b/AR1
  scalar  = half the A^T/Y drains, softmax activations, cc_out loads
  vector  = bf16 converts, other drains, softmax vector ops
"""

import numpy as np
from contextlib import ExitStack

import concourse.bass as bass
import concourse.tile as tile
from concourse import bacc, mybir
from concourse.bass import ts
from concourse.bass_utils import run_bass_kernel_spmd
from concourse.masks import make_identity

F32 = mybir.dt.float32
BF16 = mybir.dt.bfloat16
AF = mybir.ActivationFunctionType

N_CORES = 8
B = 2
L = 48 * 48 * 48          # 110592
C = 256
L_SH = L // N_CORES       # 13824 rows per core per batch
ROWS = B * L_SH           # 27648 rows per core
P = 128
RPP = 12                  # rows per partition per supertile
HPP = RPP // 2            # rows per partition per half-supertile
SROWS = P * RPP           # 1536 rows per supertile
HROWS = P * HPP           # 768 rows per half-supertile
SPB = L_SH // SROWS       # 9 supertiles per batch
S_TOT = B * SPB           # 18 supertiles per core
SPLIT0 = 3                # supertiles 0..2 -> AR0a, 3..8 -> AR0b

DEFERRED = {7, 8, 16, 17}  # transposes run in the AR wait gaps

_CACHE: dict = {}


def _build():
    nc = bacc.Bacc(
        "TRN2", target_bir_lowering=False, debug=False, num_devices=N_CORES
    )
    x_dram = nc.dram_tensor("x", [ROWS, C], F32, kind="ExternalInput")
    g_dram = nc.dram_tensor("gamma", [1, 1], F32, kind="ExternalInput")
    o_dram = nc.dram_tensor("out", [ROWS, C], F32, kind="ExternalOutput")
    # three collective groups: 0 = b0 supertiles 0..4, 1 = b0 5..8, 2 = b1
    cc_in = [
        nc.dram_tensor(f"cc_in{g}", [2 * P, C], BF16, kind="Internal")
        for g in range(3)
    ]
    cc_out = [
        nc.dram_tensor(
            f"cc_out{g}", [2 * P, C], BF16, kind="Internal",
            addr_space="Shared",
        )
        for g in range(3)
    ]
    X, GAM, OUT = x_dram.ap(), g_dram.ap(), o_dram.ap()

    def x_half(s, h):
        # partition p holds rows s*SROWS + h*HROWS + p*HPP + (0..HPP-1)
        r0 = s * SROWS + h * HROWS
        return X[r0 : r0 + HROWS, :].rearrange("(p j) c -> p j c", j=HPP)

    def x_super(s):
        # batch-1 mapping: partition p holds rows s*SROWS + p*RPP + (0..RPP-1)
        # (contiguous 12KB per partition -> full-rate SWDGE cast DMA)
        return X[ts(s, SROWS), :].rearrange("(p j) c -> p j c", j=RPP)

    def o_half(s, h):
        if s < SPB:
            r0 = s * SROWS + h * HROWS
            return OUT[r0 : r0 + HROWS, :].rearrange("(p j) c -> p j c", j=HPP)
        # batch-1 row mapping matches x_super: j here is the full-tile index
        return OUT[ts(s, SROWS), :].rearrange("(p j) c -> p j c", j=RPP)[
            :, ts(h, HPP), :
        ]

    def grp(s):
        return 0 if s < SPLIT0 else (1 if s < SPB else 2)

    GRP_BOUNDS = {0: (0, SPLIT0 - 1), 1: (SPLIT0, SPB - 1), 2: (SPB, S_TOT - 1)}

    with tile.TileContext(nc) as tc, ExitStack() as octx:
        constp = octx.enter_context(tc.tile_pool(name="const", bufs=1))
        ident = constp.tile([P, P], BF16, name="ident", tag="ident")
        make_identity(nc, ident[:])
        gam_sb = constp.tile([1, 1], F32, name="gam_sb", tag="gam_sb")
        nc.sync.dma_start(gam_sb[:], GAM[:, :])
        gam_bc = constp.tile([P, 1], F32, name="gam_bc", tag="gam_bc")
        nc.gpsimd.partition_broadcast(gam_bc[:], gam_sb[:])
        # m_bf[2b+q] = gamma * softmax(G_b)[qP:(q+1)P, :] + I-block
        m_bf = [
            constp.tile([P, C], BF16, name=f"mbf{i}", tag=f"mbf{i}")
            for i in range(4)
        ]

        attp = octx.enter_context(tc.tile_pool(name="att", bufs=S_TOT))
        xbsp = octx.enter_context(tc.tile_pool(name="xbs", bufs=3))   # stream
        xbhp = octx.enter_context(tc.tile_pool(name="xbh", bufs=4))   # held
        xfp = octx.enter_context(tc.tile_pool(name="xf", bufs=4))     # fp32
        otp = octx.enter_context(tc.tile_pool(name="ot", bufs=3))
        gsp = octx.enter_context(tc.tile_pool(name="gs", bufs=6))
        smp = octx.enter_context(tc.tile_pool(name="smx", bufs=2))
        psg = octx.enter_context(tc.tile_pool(name="psg", bufs=1, space="PSUM"))
        pst = octx.enter_context(tc.tile_pool(name="pst", bufs=2, space="PSUM"))
        psy = octx.enter_context(tc.tile_pool(name="psy", bufs=3, space="PSUM"))

        # one [P, 2, C] accumulator pair per collective group
        g_pair = [
            psg.tile([P, 2, C], F32, name=f"gpr{g}", tag=f"gpr{g}")
            for g in range(3)
        ]
        att: dict = {}
        xbs: dict = {}

        def xb_tile(s):
            pool = xbhp if s in DEFERRED else xbsp
            xb = pool.tile([P, RPP, C], BF16, name="xb", tag="xb")
            xbs[s] = xb
            return xb

        def gram(s, h):
            g, xb = grp(s), xbs[s]
            s_first, s_last = GRP_BOUNDS[g]
            for jj in range(HPP):
                j = h * HPP + jj
                first = s == s_first and j == 0
                last = s == s_last and j == RPP - 1
                nc.tensor.matmul(
                    g_pair[g][:, 0, :], xb[:, j, 0:P], xb[:, j, :],
                    start=first, stop=last,
                )
                nc.tensor.matmul(
                    g_pair[g][:, 1, :], xb[:, j, P:C], xb[:, j, :],
                    start=first, stop=last,
                )

        def tp_work(s, jplo, jphi):
            xb = xbs[s]
            if s in att:
                at = att[s]
            else:
                at = attp.tile([P, RPP, 2, P], BF16, name="at", tag="at")
                att[s] = at
            for jp in range(jplo, jphi):
                tpt = pst.tile([P, 2, 2, P], F32, name="tp", tag="tp")
                for dj in range(2):
                    j = 2 * jp + dj
                    for q in range(2):
                        nc.tensor.matmul(
                            tpt[:, dj, q, :], xb[:, j, ts(q, P)], ident[:],
                            start=True, stop=True,
                        )
                dst = at[:, ts(jp, 2), :, :]
                if jp == 3:
                    nc.vector.tensor_copy(dst, tpt[:])
                else:
                    nc.scalar.activation(dst, tpt[:], AF.Copy)

        def stage_and_ar(g):
            # staging copies + DMAs + collective all on the gpsimd queue:
            # nothing else lives there, so triggers are never queue-delayed
            for m in range(2):
                gsb = gsp.tile([P, C], BF16, name="gsb", tag="gsb")
                nc.vector.tensor_copy(gsb[:], g_pair[g][:, m, :])
                nc.gpsimd.dma_start(cc_in[g].ap()[ts(m, P), :], gsb[:])
            nc.gpsimd.collective_compute(
                "AllReduce",
                mybir.AluOpType.add,
                replica_groups=[list(range(N_CORES))],
                ins=[cc_in[g].ap()[:, :]],
                outs=[cc_out[g].ap()[:, :]],
            )

        def softmax(b):
            # b0: G = cc_out0 + cc_out1; b1: G = cc_out2
            groups = [0, 1] if b == 0 else [2]
            for m in range(2):
                i = 2 * b + m
                gfs = []
                for g in groups:
                    gf = smp.tile([P, C], BF16, name=f"gf{g}", tag=f"gf{g}")
                    nc.scalar.dma_start(gf[:], cc_out[g].ap()[ts(m, P), :])
                    gfs.append(gf)
                if len(gfs) == 2:
                    gt = smp.tile([P, C], F32, name="gt", tag="gt")
                    nc.vector.tensor_add(gt[:], gfs[0][:], gfs[1][:])
                else:
                    gt = gfs[0]
                nmx = smp.tile([P, 1], F32, name="nmx", tag="nmx")
                nc.vector.tensor_reduce(
                    nmx[:], gt[:],
                    axis=mybir.AxisListType.X,
                    op=mybir.AluOpType.max,
                    negate=True,
                )
                ex = smp.tile([P, C], F32, name="ex", tag="ex")
                ssum = smp.tile([P, 1], F32, name="ssum", tag="ssum")
                nc.scalar.activation(
                    ex[:], gt[:], AF.Exp, bias=nmx[:], scale=1.0,
                    accum_out=ssum[:],
                )
                inv = smp.tile([P, 1], F32, name="inv", tag="inv")
                nc.vector.reciprocal(inv[:], ssum[:])
                sc = smp.tile([P, 1], F32, name="sc", tag="sc")
                nc.vector.tensor_mul(sc[:], inv[:], gam_bc[:])
                nc.scalar.activation(m_bf[i][:], ex[:], AF.Copy, scale=sc[:])
                # fold the residual: M = gamma*S + I (diagonal block m)
                nc.vector.tensor_add(
                    m_bf[i][:, ts(m, P)], m_bf[i][:, ts(m, P)], ident[:]
                )

        def ywork(s):
            b, at = s // SPB, att[s]
            for h in range(2):
                ot = otp.tile([P, HPP, C], F32, name="ot", tag="ot")
                for jj in range(HPP // 2):
                    jp = h * (HPP // 2) + jj
                    y = psy.tile([P, 2, C], F32, name="y", tag="y")
                    for q in range(2):
                        j = 2 * jp + q
                        nc.tensor.matmul(
                            y[:, q, :], at[:, j, 0, :], m_bf[2 * b][:],
                            start=True, stop=False,
                        )
                        nc.tensor.matmul(
                            y[:, q, :], at[:, j, 1, :], m_bf[2 * b + 1][:],
                            start=False, stop=True,
                        )
                    dst = ot[:, ts(jj, 2), :]
                    if jj == 1:
                        nc.scalar.activation(dst, y[:], AF.Copy)
                    else:
                        nc.vector.tensor_copy(dst, y[:])
                nc.sync.dma_start(o_half(s, h), ot[:])

        # ---------------- phase 1: batch 0 (sync fp32 + converts) -------
        for s in range(SPB):
            xb = xb_tile(s)
            for h in range(2):
                xf = xfp.tile([P, HPP, C], F32, name="xf", tag="xf")
                nc.sync.dma_start(xf[:], x_half(s, h))
                nc.vector.tensor_copy(xb[:, ts(h, HPP), :], xf[:])
                gram(s, h)
                if s not in DEFERRED:
                    tp_work(s, h * (HPP // 2), (h + 1) * (HPP // 2))
            if s == SPLIT0 - 1:
                stage_and_ar(0)     # AR0a: triggers ~25us, soaks start skew

        # ---------------- phase 1: batch 1 (gpsimd cast loads) ----------
        # emitted AFTER AR0a on the gpsimd queue: the queue unblocks at
        # AR0a's ENTRY (a globally synced event), so every core's batch-1
        # read stream starts in lockstep and AR1's staging - and therefore
        # its entry barrier - no longer re-pays the start skew.
        for s in range(SPB, S_TOT):
            xb = xb_tile(s)
            nc.gpsimd.dma_start(xb[:], x_super(s))  # SWDGE cast f32->bf16
            for h in range(2):
                gram(s, h)
                if s not in DEFERRED:
                    tp_work(s, h * (HPP // 2), (h + 1) * (HPP // 2))
            if s == SPB:
                stage_and_ar(1)     # AR0b: rides AR0a's synced entry
            if s == S_TOT - 1:
                stage_and_ar(2)     # AR1: staging now skew-free

        # ---------------- phase 2 ----------------
        tp_work(7, 0, RPP // 2)     # fills the AR0b wait gap
        tp_work(8, 0, RPP // 2)
        softmax(0)
        for s in range(SPB):
            ywork(s)
        tp_work(16, 0, RPP // 2)    # fills the AR1 wait gap
        tp_work(17, 0, RPP // 2)
        softmax(1)
        for s in range(SPB, S_TOT):
            ywork(s)

    nc.compile()
    return nc


def _get_nc():
    if "nc" not in _CACHE:
        _CACHE["nc"] = _build()
    return _CACHE["nc"]


def kernel(x: np.ndarray, gamma: np.ndarray, **_kw) -> np.ndarray:
    nc = _get_nc()
    x = np.asarray(x, dtype=np.float32)
    orig_shape = x.shape
    x3 = x.reshape(B, L, C)
    gam = np.asarray(gamma, dtype=np.float32).reshape(1, 1)
    in_maps = []
    for k in range(N_CORES):
        shard = np.ascontiguousarray(
            x3[:, k * L_SH : (k + 1) * L_SH, :]
        ).reshape(ROWS, C)
        in_maps.append({"x": shard, "gamma": gam})
    res = run_bass_kernel_spmd(nc, in_maps, core_ids=list(range(N_CORES)))
    out = np.empty((B, L, C), dtype=np.float32)
    for k in range(N_CORES):
        out[:, k * L_SH : (k + 1) * L_SH, :] = res.results[k]["out"].reshape(
            B, L_SH, C
        )
    return out.reshape(orig_shape)


# revision 16
# speedup vs baseline: 1.2595x; 1.2183x over previous
"""Channel-attention (CAM) kernel for Trainium2, 8 NeuronCores.

Reference computation (per batch b):
    A   = x[b].reshape(L, C)            # L = 48^3 = 110592, C = 256
    G   = A^T A                          # [C, C] Gram matrix
    S   = softmax(G, axis=-1)
    out = gamma * (A @ S) + x[b]

Algebraic fold: out = A @ (gamma*S + I) since A @ I == x.  This removes
the residual add AND the second read of x: HBM traffic is the floor
(read 28.3 MB + write 28.3 MB per core).  A^T (bf16) stays resident in
SBUF between the phases.

Collective-latency engineering (the dominant cost): an AllReduce here
completes ~16-20 us after the LAST rank triggers it, and the runtime
starts the 8 cores with a 20-30 us skew, so the first collective also
absorbs that skew in its entry barrier.  Batch-0's Gram is therefore
all-reduced in TWO pieces: AR0a covers supertiles 0-4 and triggers at
~35 us (soaking the start skew while the read stream continues), AR0b
covers 5-8 and triggers as soon as AR0a completes (ranks now aligned,
so it costs only the ~16-20 us floor); G_0 = out_a + out_b summed
locally inside the softmax.  AR1 (batch 1, one piece) then rides the
aligned ranks too and overlaps batch-0's Y matmuls + stores.

The gpsimd queue carries ONLY Gram staging + the collectives (a
collective blocks its queue until completion, so nothing else may sit
behind one).  All x loads are plain fp32 halves on the sync queue,
converted to bf16 on the vector engine.  Transposes for 3 supertiles
are deferred into the AR wait gaps so phase-1 PE work stays under the
DMA read stream.

Engine queues (FIFO, emission order = issue order):
  sync    = gamma, all x loads, output stores
  gpsimd  = Gram staging copies + staging DMAs + AR0a/AR0b/AR1
  scalar  = half the A^T/Y drains, softmax activations, cc_out loads
  vector  = bf16 converts, other drains, softmax vector ops
"""

import numpy as np
from contextlib import ExitStack

import concourse.bass as bass
import concourse.tile as tile
from concourse import bacc, mybir
from concourse.bass import ts
from concourse.bass_utils import run_bass_kernel_spmd
from concourse.masks import make_identity

F32 = mybir.dt.float32
BF16 = mybir.dt.bfloat16
AF = mybir.ActivationFunctionType

N_CORES = 8
B = 2
L = 48 * 48 * 48          # 110592
C = 256
L_SH = L // N_CORES       # 13824 rows per core per batch
ROWS = B * L_SH           # 27648 rows per core
P = 128
RPP = 12                  # rows per partition per supertile
HPP = RPP // 2            # rows per partition per half-supertile
SROWS = P * RPP           # 1536 rows per supertile
HROWS = P * HPP           # 768 rows per half-supertile
SPB = L_SH // SROWS       # 9 supertiles per batch
S_TOT = B * SPB           # 18 supertiles per core
SPLIT0 = 3                # supertiles 0..2 -> AR0a, 3..8 -> AR0b

DEFERRED = {8, 16, 17}    # transposes run in the AR wait gaps

_CACHE: dict = {}


def _build():
    nc = bacc.Bacc(
        "TRN2", target_bir_lowering=False, debug=False, num_devices=N_CORES
    )
    x_dram = nc.dram_tensor("x", [ROWS, C], F32, kind="ExternalInput")
    g_dram = nc.dram_tensor("gamma", [1, 1], F32, kind="ExternalInput")
    o_dram = nc.dram_tensor("out", [ROWS, C], F32, kind="ExternalOutput")
    # three collective groups: 0 = b0 supertiles 0..4, 1 = b0 5..8, 2 = b1
    cc_in = [
        nc.dram_tensor(f"cc_in{g}", [2 * P, C], BF16, kind="Internal")
        for g in range(3)
    ]
    cc_out = [
        nc.dram_tensor(
            f"cc_out{g}", [2 * P, C], BF16, kind="Internal",
            addr_space="Shared",
        )
        for g in range(3)
    ]
    X, GAM, OUT = x_dram.ap(), g_dram.ap(), o_dram.ap()

    def x_half(s, h):
        # partition p holds rows s*SROWS + h*HROWS + p*HPP + (0..HPP-1)
        r0 = s * SROWS + h * HROWS
        return X[r0 : r0 + HROWS, :].rearrange("(p j) c -> p j c", j=HPP)

    def o_half(s, h):
        r0 = s * SROWS + h * HROWS
        return OUT[r0 : r0 + HROWS, :].rearrange("(p j) c -> p j c", j=HPP)

    def grp(s):
        return 0 if s < SPLIT0 else (1 if s < SPB else 2)

    GRP_BOUNDS = {0: (0, SPLIT0 - 1), 1: (SPLIT0, SPB - 1), 2: (SPB, S_TOT - 1)}

    with tile.TileContext(nc) as tc, ExitStack() as octx:
        constp = octx.enter_context(tc.tile_pool(name="const", bufs=1))
        ident = constp.tile([P, P], BF16, name="ident", tag="ident")
        make_identity(nc, ident[:])
        gam_sb = constp.tile([1, 1], F32, name="gam_sb", tag="gam_sb")
        nc.sync.dma_start(gam_sb[:], GAM[:, :])
        gam_bc = constp.tile([P, 1], F32, name="gam_bc", tag="gam_bc")
        nc.gpsimd.partition_broadcast(gam_bc[:], gam_sb[:])
        # m_bf[2b+q] = gamma * softmax(G_b)[qP:(q+1)P, :] + I-block
        m_bf = [
            constp.tile([P, C], BF16, name=f"mbf{i}", tag=f"mbf{i}")
            for i in range(4)
        ]

        attp = octx.enter_context(tc.tile_pool(name="att", bufs=S_TOT))
        xbsp = octx.enter_context(tc.tile_pool(name="xbs", bufs=3))   # stream
        xbhp = octx.enter_context(tc.tile_pool(name="xbh", bufs=3))   # held
        xfp = octx.enter_context(tc.tile_pool(name="xf", bufs=6))     # fp32
        otp = octx.enter_context(tc.tile_pool(name="ot", bufs=3))
        gsp = octx.enter_context(tc.tile_pool(name="gs", bufs=4))
        smp = octx.enter_context(tc.tile_pool(name="smx", bufs=1))
        psg = octx.enter_context(tc.tile_pool(name="psg", bufs=1, space="PSUM"))
        pst = octx.enter_context(tc.tile_pool(name="pst", bufs=2, space="PSUM"))
        psy = octx.enter_context(tc.tile_pool(name="psy", bufs=3, space="PSUM"))

        # one [P, 2, C] accumulator pair per collective group
        g_pair = [
            psg.tile([P, 2, C], F32, name=f"gpr{g}", tag=f"gpr{g}")
            for g in range(3)
        ]
        att: dict = {}
        xbs: dict = {}

        def xb_tile(s):
            pool = xbhp if s in DEFERRED else xbsp
            xb = pool.tile([P, RPP, C], BF16, name="xb", tag="xb")
            xbs[s] = xb
            return xb

        def gram(s, h):
            g, xb = grp(s), xbs[s]
            s_first, s_last = GRP_BOUNDS[g]
            for jj in range(HPP):
                j = h * HPP + jj
                first = s == s_first and j == 0
                last = s == s_last and j == RPP - 1
                nc.tensor.matmul(
                    g_pair[g][:, 0, :], xb[:, j, 0:P], xb[:, j, :],
                    start=first, stop=last,
                )
                nc.tensor.matmul(
                    g_pair[g][:, 1, :], xb[:, j, P:C], xb[:, j, :],
                    start=first, stop=last,
                )

        def tp_work(s, jplo, jphi):
            xb = xbs[s]
            if s in att:
                at = att[s]
            else:
                at = attp.tile([P, RPP, 2, P], BF16, name="at", tag="at")
                att[s] = at
            for jp in range(jplo, jphi):
                tpt = pst.tile([P, 2, 2, P], F32, name="tp", tag="tp")
                for dj in range(2):
                    j = 2 * jp + dj
                    for q in range(2):
                        nc.tensor.matmul(
                            tpt[:, dj, q, :], xb[:, j, ts(q, P)], ident[:],
                            start=True, stop=True,
                        )
                dst = at[:, ts(jp, 2), :, :]
                if jp == 3:
                    nc.vector.tensor_copy(dst, tpt[:])
                else:
                    nc.scalar.activation(dst, tpt[:], AF.Copy)

        def stage_and_ar(g):
            # staging copies + DMAs + collective all on the gpsimd queue:
            # nothing else lives there, so triggers are never queue-delayed
            for m in range(2):
                gsb = gsp.tile([P, C], BF16, name="gsb", tag="gsb")
                nc.vector.tensor_copy(gsb[:], g_pair[g][:, m, :])
                nc.gpsimd.dma_start(cc_in[g].ap()[ts(m, P), :], gsb[:])
            nc.gpsimd.collective_compute(
                "AllReduce",
                mybir.AluOpType.add,
                replica_groups=[list(range(N_CORES))],
                ins=[cc_in[g].ap()[:, :]],
                outs=[cc_out[g].ap()[:, :]],
            )

        def softmax(b):
            # b0: G = cc_out0 + cc_out1; b1: G = cc_out2
            groups = [0, 1] if b == 0 else [2]
            for m in range(2):
                i = 2 * b + m
                gfs = []
                for g in groups:
                    gf = smp.tile([P, C], BF16, name=f"gf{g}", tag=f"gf{g}")
                    nc.scalar.dma_start(gf[:], cc_out[g].ap()[ts(m, P), :])
                    gfs.append(gf)
                if len(gfs) == 2:
                    gt = smp.tile([P, C], F32, name="gt", tag="gt")
                    nc.vector.tensor_add(gt[:], gfs[0][:], gfs[1][:])
                else:
                    gt = gfs[0]
                nmx = smp.tile([P, 1], F32, name="nmx", tag="nmx")
                nc.vector.tensor_reduce(
                    nmx[:], gt[:],
                    axis=mybir.AxisListType.X,
                    op=mybir.AluOpType.max,
                    negate=True,
                )
                ex = smp.tile([P, C], F32, name="ex", tag="ex")
                ssum = smp.tile([P, 1], F32, name="ssum", tag="ssum")
                nc.scalar.activation(
                    ex[:], gt[:], AF.Exp, bias=nmx[:], scale=1.0,
                    accum_out=ssum[:],
                )
                inv = smp.tile([P, 1], F32, name="inv", tag="inv")
                nc.vector.reciprocal(inv[:], ssum[:])
                sc = smp.tile([P, 1], F32, name="sc", tag="sc")
                nc.vector.tensor_mul(sc[:], inv[:], gam_bc[:])
                nc.scalar.activation(m_bf[i][:], ex[:], AF.Copy, scale=sc[:])
                # fold the residual: M = gamma*S + I (diagonal block m)
                nc.vector.tensor_add(
                    m_bf[i][:, ts(m, P)], m_bf[i][:, ts(m, P)], ident[:]
                )

        def ywork(s):
            b, at = s // SPB, att[s]
            for h in range(2):
                ot = otp.tile([P, HPP, C], F32, name="ot", tag="ot")
                for jj in range(HPP // 2):
                    jp = h * (HPP // 2) + jj
                    y = psy.tile([P, 2, C], F32, name="y", tag="y")
                    for q in range(2):
                        j = 2 * jp + q
                        nc.tensor.matmul(
                            y[:, q, :], at[:, j, 0, :], m_bf[2 * b][:],
                            start=True, stop=False,
                        )
                        nc.tensor.matmul(
                            y[:, q, :], at[:, j, 1, :], m_bf[2 * b + 1][:],
                            start=False, stop=True,
                        )
                    dst = ot[:, ts(jj, 2), :]
                    if jj == 1:
                        nc.scalar.activation(dst, y[:], AF.Copy)
                    else:
                        nc.vector.tensor_copy(dst, y[:])
                nc.sync.dma_start(o_half(s, h), ot[:])

        # ---------------- phase 1 ----------------
        for s in range(S_TOT):
            xb = xb_tile(s)
            for h in range(2):
                xf = xfp.tile([P, HPP, C], F32, name="xf", tag="xf")
                nc.sync.dma_start(xf[:], x_half(s, h))
                nc.vector.tensor_copy(xb[:, ts(h, HPP), :], xf[:])
                gram(s, h)
                if s not in DEFERRED:
                    tp_work(s, h * (HPP // 2), (h + 1) * (HPP // 2))
            if s == SPLIT0 - 1:
                stage_and_ar(0)     # AR0a: triggers ~35us, soaks start skew
            if s == SPB - 1:
                stage_and_ar(1)     # AR0b: triggers when AR0a completes
            if s == S_TOT - 1:
                stage_and_ar(2)     # AR1: trigger rides its own staging

        # ---------------- phase 2 ----------------
        tp_work(8, 0, RPP // 2)     # fills the AR0b wait gap
        softmax(0)
        for s in range(SPB):
            ywork(s)
        tp_work(16, 0, RPP // 2)    # fills the AR1 wait gap
        tp_work(17, 0, RPP // 2)
        softmax(1)
        for s in range(SPB, S_TOT):
            ywork(s)

    nc.compile()
    return nc


def _get_nc():
    if "nc" not in _CACHE:
        _CACHE["nc"] = _build()
    return _CACHE["nc"]


def kernel(x: np.ndarray, gamma: np.ndarray, **_kw) -> np.ndarray:
    nc = _get_nc()
    x = np.asarray(x, dtype=np.float32)
    orig_shape = x.shape
    x3 = x.reshape(B, L, C)
    gam = np.asarray(gamma, dtype=np.float32).reshape(1, 1)
    in_maps = []
    for k in range(N_CORES):
        shard = np.ascontiguousarray(
            x3[:, k * L_SH : (k + 1) * L_SH, :]
        ).reshape(ROWS, C)
        in_maps.append({"x": shard, "gamma": gam})
    res = run_bass_kernel_spmd(nc, in_maps, core_ids=list(range(N_CORES)))
    out = np.empty((B, L, C), dtype=np.float32)
    for k in range(N_CORES):
        out[:, k * L_SH : (k + 1) * L_SH, :] = res.results[k]["out"].reshape(
            B, L_SH, C
        )
    return out.reshape(orig_shape)


# revision 19
# speedup vs baseline: 1.4771x; 1.1727x over previous
"""Channel-attention (CAM) kernel for Trainium2, 8 NeuronCores.

Reference computation (per batch b):
    A   = x[b].reshape(L, C)            # L = 48^3 = 110592, C = 256
    G   = A^T A                          # [C, C] Gram matrix
    S   = softmax(G, axis=-1)
    out = gamma * (A @ S) + x[b]

Algebraic fold: out = A @ (gamma*S + I) since A @ I == x.  This removes
the residual add AND the second read of x: HBM traffic is the floor
(read 28.3 MB + write 28.3 MB per core).  A^T (bf16) stays resident in
SBUF between the phases.

Collective-latency engineering (the dominant cost): an AllReduce here
completes ~16-20 us after the LAST rank triggers it, and the runtime
starts the 8 cores with a 20-30 us skew, so the first collective also
absorbs that skew in its entry barrier.  Batch-0's Gram is therefore
all-reduced in TWO pieces: AR0a covers supertiles 0-4 and triggers at
~35 us (soaking the start skew while the read stream continues), AR0b
covers 5-8 and triggers as soon as AR0a completes (ranks now aligned,
so it costs only the ~16-20 us floor); G_0 = out_a + out_b summed
locally inside the softmax.  AR1 (batch 1, one piece) then rides the
aligned ranks too and overlaps batch-0's Y matmuls + stores.

The gpsimd queue carries ONLY Gram staging + the collectives (a
collective blocks its queue until completion, so nothing else may sit
behind one).  All x loads are plain fp32 halves on the sync queue,
converted to bf16 on the vector engine.  Transposes for 3 supertiles
are deferred into the AR wait gaps so phase-1 PE work stays under the
DMA read stream.

Engine queues (FIFO, emission order = issue order):
  sync    = gamma, all x loads, output stores
  gpsimd  = Gram staging copies + staging DMAs + AR0a/AR0b/AR1
  scalar  = half the A^T/Y drains, softmax activations, cc_out loads
  vector  = bf16 converts, other drains, softmax vector ops
"""

import numpy as np
from contextlib import ExitStack

import concourse.bass as bass
import concourse.tile as tile
from concourse import bacc, mybir
from concourse.bass import ts
from concourse.bass_utils import run_bass_kernel_spmd
from concourse.masks import make_identity

F32 = mybir.dt.float32
BF16 = mybir.dt.bfloat16
AF = mybir.ActivationFunctionType

N_CORES = 8
B = 2
L = 48 * 48 * 48          # 110592
C = 256
L_SH = L // N_CORES       # 13824 rows per core per batch
ROWS = B * L_SH           # 27648 rows per core
P = 128
RPP = 12                  # rows per partition per supertile
HPP = RPP // 2            # rows per partition per half-supertile
SROWS = P * RPP           # 1536 rows per supertile
HROWS = P * HPP           # 768 rows per half-supertile
SPB = L_SH // SROWS       # 9 supertiles per batch
S_TOT = B * SPB           # 18 supertiles per core
SPLIT0 = 3                # supertiles 0..2 -> AR0a, 3..8 -> AR0b

DEFERRED = {8, 16, 17}    # transposes run in the AR wait gaps

_CACHE: dict = {}


def _build():
    nc = bacc.Bacc(
        "TRN2", target_bir_lowering=False, debug=False, num_devices=N_CORES
    )
    x_dram = nc.dram_tensor("x", [ROWS, C], F32, kind="ExternalInput")
    g_dram = nc.dram_tensor("gamma", [1, 1], F32, kind="ExternalInput")
    o_dram = nc.dram_tensor("out", [ROWS, C], F32, kind="ExternalOutput")
    # three collective groups: 0 = b0 supertiles 0..4, 1 = b0 5..8, 2 = b1
    cc_in = [
        nc.dram_tensor(f"cc_in{g}", [2 * P, C], BF16, kind="Internal")
        for g in range(3)
    ]
    cc_out = [
        nc.dram_tensor(
            f"cc_out{g}", [2 * P, C], BF16, kind="Internal",
            addr_space="Shared",
        )
        for g in range(3)
    ]
    X, GAM, OUT = x_dram.ap(), g_dram.ap(), o_dram.ap()

    def x_half(s, h):
        # partition p holds rows s*SROWS + h*HROWS + p*HPP + (0..HPP-1)
        r0 = s * SROWS + h * HROWS
        return X[r0 : r0 + HROWS, :].rearrange("(p j) c -> p j c", j=HPP)

    def o_half(s, h):
        r0 = s * SROWS + h * HROWS
        return OUT[r0 : r0 + HROWS, :].rearrange("(p j) c -> p j c", j=HPP)

    def grp(s):
        return 0 if s < SPLIT0 else (1 if s < SPB else 2)

    GRP_BOUNDS = {0: (0, SPLIT0 - 1), 1: (SPLIT0, SPB - 1), 2: (SPB, S_TOT - 1)}

    with tile.TileContext(nc) as tc, ExitStack() as octx:
        constp = octx.enter_context(tc.tile_pool(name="const", bufs=1))
        ident = constp.tile([P, P], BF16, name="ident", tag="ident")
        make_identity(nc, ident[:])
        gam_sb = constp.tile([1, 1], F32, name="gam_sb", tag="gam_sb")
        nc.sync.dma_start(gam_sb[:], GAM[:, :])
        gam_bc = constp.tile([P, 1], F32, name="gam_bc", tag="gam_bc")
        nc.gpsimd.partition_broadcast(gam_bc[:], gam_sb[:])
        # m_bf[2b+q] = gamma * softmax(G_b)[qP:(q+1)P, :] + I-block
        m_bf = [
            constp.tile([P, C], BF16, name=f"mbf{i}", tag=f"mbf{i}")
            for i in range(4)
        ]

        attp = octx.enter_context(tc.tile_pool(name="att", bufs=S_TOT))
        xbsp = octx.enter_context(tc.tile_pool(name="xbs", bufs=3))   # stream
        xbhp = octx.enter_context(tc.tile_pool(name="xbh", bufs=3))   # held
        xfp = octx.enter_context(tc.tile_pool(name="xf", bufs=6))     # fp32
        otp = octx.enter_context(tc.tile_pool(name="ot", bufs=3))
        gsp = octx.enter_context(tc.tile_pool(name="gs", bufs=3))
        smp = octx.enter_context(tc.tile_pool(name="smx", bufs=1))
        psg = octx.enter_context(tc.tile_pool(name="psg", bufs=1, space="PSUM"))
        pst = octx.enter_context(tc.tile_pool(name="pst", bufs=2, space="PSUM"))
        psy = octx.enter_context(tc.tile_pool(name="psy", bufs=3, space="PSUM"))

        # one [P, 2, C] accumulator pair per collective group
        g_pair = [
            psg.tile([P, 2, C], F32, name=f"gpr{g}", tag=f"gpr{g}")
            for g in range(3)
        ]
        att: dict = {}
        xbs: dict = {}

        def xb_tile(s):
            pool = xbhp if s in DEFERRED else xbsp
            xb = pool.tile([P, RPP, C], BF16, name="xb", tag="xb")
            xbs[s] = xb
            return xb

        def gram(s, h):
            g, xb = grp(s), xbs[s]
            s_first, s_last = GRP_BOUNDS[g]
            for jj in range(HPP):
                j = h * HPP + jj
                first = s == s_first and j == 0
                last = s == s_last and j == RPP - 1
                nc.tensor.matmul(
                    g_pair[g][:, 0, :], xb[:, j, 0:P], xb[:, j, :],
                    start=first, stop=last,
                )
                nc.tensor.matmul(
                    g_pair[g][:, 1, :], xb[:, j, P:C], xb[:, j, :],
                    start=first, stop=last,
                )

        def tp_work(s, jplo, jphi):
            xb = xbs[s]
            if s in att:
                at = att[s]
            else:
                at = attp.tile([P, RPP, 2, P], BF16, name="at", tag="at")
                att[s] = at
            for jp in range(jplo, jphi):
                tpt = pst.tile([P, 2, 2, P], F32, name="tp", tag="tp")
                for dj in range(2):
                    j = 2 * jp + dj
                    for q in range(2):
                        nc.tensor.matmul(
                            tpt[:, dj, q, :], xb[:, j, ts(q, P)], ident[:],
                            start=True, stop=True,
                        )
                dst = at[:, ts(jp, 2), :, :]
                if jp == 3:
                    nc.vector.tensor_copy(dst, tpt[:])
                else:
                    nc.scalar.activation(dst, tpt[:], AF.Copy)

        def stage_and_ar(g):
            # staging copies + DMAs + collective all on the gpsimd queue:
            # nothing else lives there, so triggers are never queue-delayed
            for m in range(2):
                gsb = gsp.tile([P, C], BF16, name="gsb", tag="gsb")
                nc.vector.tensor_copy(gsb[:], g_pair[g][:, m, :])
                nc.gpsimd.dma_start(cc_in[g].ap()[ts(m, P), :], gsb[:])
            nc.gpsimd.collective_compute(
                "AllReduce",
                mybir.AluOpType.add,
                replica_groups=[list(range(N_CORES))],
                ins=[cc_in[g].ap()[:, :]],
                outs=[cc_out[g].ap()[:, :]],
            )

        def softmax(b):
            # b0: G = cc_out0 + cc_out1; b1: G = cc_out2
            groups = [0, 1] if b == 0 else [2]
            for m in range(2):
                i = 2 * b + m
                gfs = []
                for g in groups:
                    gf = smp.tile([P, C], BF16, name=f"gf{g}", tag=f"gf{g}")
                    nc.scalar.dma_start(gf[:], cc_out[g].ap()[ts(m, P), :])
                    gfs.append(gf)
                if len(gfs) == 2:
                    gt = smp.tile([P, C], F32, name="gt", tag="gt")
                    nc.vector.tensor_add(gt[:], gfs[0][:], gfs[1][:])
                else:
                    gt = gfs[0]
                nmx = smp.tile([P, 1], F32, name="nmx", tag="nmx")
                nc.vector.tensor_reduce(
                    nmx[:], gt[:],
                    axis=mybir.AxisListType.X,
                    op=mybir.AluOpType.max,
                    negate=True,
                )
                ex = smp.tile([P, C], F32, name="ex", tag="ex")
                ssum = smp.tile([P, 1], F32, name="ssum", tag="ssum")
                nc.scalar.activation(
                    ex[:], gt[:], AF.Exp, bias=nmx[:], scale=1.0,
                    accum_out=ssum[:],
                )
                inv = smp.tile([P, 1], F32, name="inv", tag="inv")
                nc.vector.reciprocal(inv[:], ssum[:])
                sc = smp.tile([P, 1], F32, name="sc", tag="sc")
                nc.vector.tensor_mul(sc[:], inv[:], gam_bc[:])
                nc.scalar.activation(m_bf[i][:], ex[:], AF.Copy, scale=sc[:])
                # fold the residual: M = gamma*S + I (diagonal block m)
                nc.vector.tensor_add(
                    m_bf[i][:, ts(m, P)], m_bf[i][:, ts(m, P)], ident[:]
                )

        def ywork(s):
            b, at = s // SPB, att[s]
            for h in range(2):
                ot = otp.tile([P, HPP, C], F32, name="ot", tag="ot")
                for jj in range(HPP // 2):
                    jp = h * (HPP // 2) + jj
                    y = psy.tile([P, 2, C], F32, name="y", tag="y")
                    for q in range(2):
                        j = 2 * jp + q
                        nc.tensor.matmul(
                            y[:, q, :], at[:, j, 0, :], m_bf[2 * b][:],
                            start=True, stop=False,
                        )
                        nc.tensor.matmul(
                            y[:, q, :], at[:, j, 1, :], m_bf[2 * b + 1][:],
                            start=False, stop=True,
                        )
                    dst = ot[:, ts(jj, 2), :]
                    if jj == 1:
                        nc.scalar.activation(dst, y[:], AF.Copy)
                    else:
                        nc.vector.tensor_copy(dst, y[:])
                nc.sync.dma_start(o_half(s, h), ot[:])

        # ---------------- phase 1 ----------------
        for s in range(S_TOT):
            xb = xb_tile(s)
            for h in range(2):
                xf = xfp.tile([P, HPP, C], F32, name="xf", tag="xf")
                nc.sync.dma_start(xf[:], x_half(s, h))
                nc.vector.tensor_copy(xb[:, ts(h, HPP), :], xf[:])
                gram(s, h)
                if s not in DEFERRED:
                    tp_work(s, h * (HPP // 2), (h + 1) * (HPP // 2))
            if s == SPLIT0 - 1:
                stage_and_ar(0)     # AR0a: triggers ~35us, soaks start skew
            if s == SPB - 1:
                stage_and_ar(1)     # AR0b: triggers when AR0a completes
            if s == S_TOT - 1:
                # skew insurance: gate AR1's trigger on AR0b COMPLETION (a
                # globally synced event) instead of core-local staging, so
                # its entry barrier doesn't re-pay the start skew
                ccw = gsp.tile([1, C], BF16, name="ccw", tag="ccw")
                nc.gpsimd.dma_start(ccw[:], cc_out[1].ap()[0:1, :])
                stage_and_ar(2)     # AR1

        # ---------------- phase 2 ----------------
        tp_work(8, 0, RPP // 2)     # fills the AR0b wait gap
        softmax(0)
        for s in range(SPB):
            ywork(s)
        tp_work(16, 0, RPP // 2)    # fills the AR1 wait gap
        tp_work(17, 0, RPP // 2)
        softmax(1)
        for s in range(SPB, S_TOT):
            ywork(s)

    nc.compile()
    return nc


def _get_nc():
    if "nc" not in _CACHE:
        _CACHE["nc"] = _build()
    return _CACHE["nc"]


def kernel(x: np.ndarray, gamma: np.ndarray, **_kw) -> np.ndarray:
    nc = _get_nc()
    x = np.asarray(x, dtype=np.float32)
    orig_shape = x.shape
    x3 = x.reshape(B, L, C)
    gam = np.asarray(gamma, dtype=np.float32).reshape(1, 1)
    in_maps = []
    for k in range(N_CORES):
        shard = np.ascontiguousarray(
            x3[:, k * L_SH : (k + 1) * L_SH, :]
        ).reshape(ROWS, C)
        in_maps.append({"x": shard, "gamma": gam})
    res = run_bass_kernel_spmd(nc, in_maps, core_ids=list(range(N_CORES)))
    out = np.empty((B, L, C), dtype=np.float32)
    for k in range(N_CORES):
        out[:, k * L_SH : (k + 1) * L_SH, :] = res.results[k]["out"].reshape(
            B, L_SH, C
        )
    return out.reshape(orig_shape)
